# revision 4
# baseline (speedup 1.0000x reference)
"""ResNet BasicBlock (conv3x3-BN-ReLU-conv3x3-BN-+res-ReLU) on 8 trn2 NeuronCores.

Data-parallel over the batch (4 images per core). BatchNorm uses global batch
statistics, reduced across cores with a small AllGather.

Per-core layout: channels on partitions; partitions 0-63 hold images {0,1} of
the core's shard, partitions 64-127 images {2,3}. Each 3x3 conv is 9 shifted
matmuls accumulating in PSUM (fp32r, full column rate); the two image halves
run as concurrent row-tiled matmuls in quadrants (0,0)/(64,0) of the PE array.
Matmul outputs can only land on PSUM partitions 0-63, so half B's eviction
hops across the partition boundary with a SBUF->SBUF DMA.

Feature planes are stored 58 columns wide with zeroed border columns so the
horizontal taps stay full-width (fp32r PSUM writes need 8B-aligned offsets);
the vertical taps use valid-row ranges instead of row padding, with the
always-full center tap first in each accumulation group to clear the bank.
"""
import numpy as np
from contextlib import ExitStack

import concourse.bass as bass
import concourse.bacc as bacc
import concourse.mybir as mybir
import concourse.tile as tile
from concourse.bass_utils import run_bass_kernel_spmd

N_CORES = 8
B, C, H, W = 32, 64, 56, 56
BL = B // N_CORES           # images per core
P = 64                      # conv output channels
PW = W + 2                  # column-padded plane width
EPS = 1e-5
RB = 4                      # output rows per chunk
NCHUNK = H // RB            # 14
NFREE = 2 * RB * W          # 448 matmul moving columns
NTOT = float(B * H * W)     # BN normalization count
GRP = 4                     # chunks per psum group

f32 = mybir.dt.float32
f32r = mybir.dt.float32r
AF = mybir.ActivationFunctionType
ALU = mybir.AluOpType
AX = mybir.AxisListType

# center tap first: it is full-coverage for every chunk, so its start=True
# clears the whole PSUM bank before the partial edge taps accumulate.
TAPS = [(1, 1), (0, 0), (0, 1), (0, 2), (1, 0), (1, 2), (2, 0), (2, 1), (2, 2)]


def build(n_cores=N_CORES):
    nc = bacc.Bacc(
        "TRN2", target_bir_lowering=False, debug=False,
        enable_asserts=False, num_devices=n_cores,
    )
    xs_d = nc.dram_tensor("xs", [BL, C, H, W], f32r, kind="ExternalInput")
    w1_d = nc.dram_tensor("w1p", [9, 128, P], f32r, kind="ExternalInput")
    w2_d = nc.dram_tensor("w2p", [9, 128, P], f32r, kind="ExternalInput")
    bn1_d = nc.dram_tensor("bn1", [128, 2], f32, kind="ExternalInput")
    bn2_d = nc.dram_tensor("bn2", [128, 2], f32, kind="ExternalInput")
    out_d = nc.dram_tensor("out", [BL, C, H, W], f32, kind="ExternalOutput")

    with tile.TileContext(nc) as tc:
        with ExitStack() as ctx:
            main = ctx.enter_context(tc.tile_pool(name="main", bufs=1))
            psum = ctx.enter_context(tc.tile_pool(name="psum", bufs=1, space="PSUM"))
            hop = ctx.enter_context(tc.tile_pool(name="hop", bufs=1))
            smal = ctx.enter_context(tc.tile_pool(name="smal", bufs=1))
            dram = ctx.enter_context(tc.tile_pool(name="dram", bufs=1, space="DRAM"))

            x_sb = main.tile([128, 2, H, PW], f32r)
            z_sb = main.tile([128, 2, H, PW], f32r)
            y2 = main.tile([128, 2, H, W], f32)
            fin = main.tile([128, 2, H, W], f32)
            w1s = main.tile([128, 9, P], f32r)
            w2s = main.tile([128, 9, P], f32r)
            gb1 = main.tile([128, 2], f32)
            gb2 = main.tile([128, 2], f32)
            sp1 = main.tile([64, NCHUNK, 2, 6], f32)
            sp2 = main.tile([64, NCHUNK, 2, 6], f32)

            # ACT table preload (sqrt set also carries relu/copy) so the
            # ~2.7us table DMA overlaps the input loads instead of landing on
            # the BN critical path.
            dumm = smal.tile([128, 1], f32, name="dumm")
            nc.vector.memset(dumm[:], 1.0)
            dum2 = smal.tile([128, 1], f32, name="dum2")
            nc.scalar.activation(dum2[:], dumm[:], AF.Sqrt)
            nc.scalar.activation(dum2[:], dumm[:], AF.Relu)

            # weights / bn params
            nc.sync.dma_start(w1s[:], w1_d[:].rearrange("t p o -> p t o"))
            nc.sync.dma_start(w2s[:], w2_d[:].rearrange("t p o -> p t o"))
            nc.sync.dma_start(gb1[:], bn1_d[:])
            nc.sync.dma_start(gb2[:], bn2_d[:])

            # zero the padding columns of both feature buffers
            for pad in (x_sb, z_sb):
                nc.vector.memset(pad[:, :, :, 0].bitcast(f32), 0.0)
                nc.vector.memset(pad[:, :, :, PW - 1].bitcast(f32), 0.0)

            # load x into the column interior, split across DMA queues
            for b in range(BL):
                hh, j = divmod(b, 2)
                for rb in range(0, H, 8):
                    nc.sync.dma_start(
                        x_sb[64 * hh:64 * hh + 64, j, rb:rb + 8, 1:1 + W],
                        xs_d[b, :, rb:rb + 8, :],
                    )

            def conv(src, wsb, evict):
                for cg0 in range(0, NCHUNK, GRP):
                    cn = min(GRP, NCHUNK - cg0)
                    pas = [psum.tile([64, 2, RB, W], f32, name="psA", tag="psA",
                                     bufs=GRP) for _ in range(cn)]
                    pbs = [psum.tile([64, 2, RB, W], f32, name="psB", tag="psB",
                                     bufs=GRP) for _ in range(cn)]
                    for ti, (ty, tx) in enumerate(TAPS):
                        dy = ty - 1
                        st = ti == 0
                        sp = ti == 8
                        for ci in range(cn):
                            r0 = RB * (cg0 + ci)
                            y0 = max(r0, -dy)
                            y1 = min(r0 + RB, H - dy)
                            il, ih = y0 - r0, y1 - r0
                            for hh, ps in ((0, pas[ci]), (1, pbs[ci])):
                                pr = 64 * hh
                                nc.tensor.matmul(
                                    ps[:, :, il:ih, :],
                                    wsb[pr:pr + 64, 3 * ty + tx, :],
                                    src[pr:pr + 64, :, y0 + dy:y1 + dy, tx:tx + W],
                                    start=st, stop=sp, tile_position=(pr, 0))
                    for ci in range(cn):
                        evict(cg0 + ci, pas[ci], pbs[ci])

            def evict1(c, pa, pb):
                r0 = RB * c
                nc.scalar.activation(
                    z_sb[0:64, :, r0:r0 + RB, 1:1 + W], pa[:], AF.Copy)
                tb = hop.tile([64, NFREE], f32, name="tb1", tag="tb", bufs=3)
                paf = pa[:].rearrange("p a b c -> p (a b c)")
                pbf = pb[:].rearrange("p a b c -> p (a b c)")
                nc.vector.tensor_copy(tb[:], pbf)
                tb4 = tb[:].rearrange("p (i r c) -> p i r c", i=2, r=RB).bitcast(f32r)
                for j in range(2):
                    nc.sync.dma_start(
                        z_sb[64:128, j, r0:r0 + RB, 1:1 + W], tb4[:, j])
                nc.vector.bn_stats(sp1[:, c, 0, :], paf)
                nc.vector.bn_stats(sp1[:, c, 1, :], pbf)

            def evict2(c, pa, pb):
                r0 = RB * c
                nc.scalar.activation(
                    y2[0:64, :, r0:r0 + RB, :], pa[:], AF.Copy)
                tb = hop.tile([64, NFREE], f32, name="tb2", tag="tb", bufs=3)
                paf = pa[:].rearrange("p a b c -> p (a b c)")
                pbf = pb[:].rearrange("p a b c -> p (a b c)")
                nc.vector.tensor_copy(tb[:], pbf)
                nc.sync.dma_start(
                    y2[64:128, :, r0:r0 + RB, :],
                    tb[:].rearrange("p (i r c) -> p i r c", i=2, r=RB))
                nc.vector.bn_stats(sp2[:, c, 0, :], paf)
                nc.vector.bn_stats(sp2[:, c, 1, :], pbf)

            def bn_sync(sparts, gb, idx):
                # fold the per-chunk bn_stats triples into local (sum, sumsq)
                t = sparts[:].rearrange("p c h (t v) -> p (c h t) v", v=3)
                nt = NCHUNK * 4
                cm = smal.tile([64, nt], f32, name=f"cm{idx}")
                nc.vector.tensor_mul(cm[:], t[:, :, 0], t[:, :, 1])
                qq = smal.tile([64, nt], f32, name=f"qq{idx}")
                nc.vector.tensor_mul(qq[:], t[:, :, 1], t[:, :, 1])
                nc.vector.tensor_mul(qq[:], qq[:], t[:, :, 0])
                nc.vector.tensor_add(qq[:], qq[:], t[:, :, 2])
                loc = smal.tile([64, 2], f32, name=f"loc{idx}")
                nc.vector.tensor_reduce(loc[:, 0:1], cm[:], axis=AX.X, op=ALU.add)
                nc.vector.tensor_reduce(loc[:, 1:2], qq[:], axis=AX.X, op=ALU.add)

                cc_in = dram.tile([64, 2], f32, name=f"ccin{idx}")
                cc_out = dram.tile([N_CORES * 64, 2], f32, name=f"ccout{idx}",
                                   addr_space="Shared")
                nc.sync.dma_start(cc_in[:], loc[:])
                nc.gpsimd.collective_compute(
                    "AllGather", ALU.bypass,
                    replica_groups=[list(range(N_CORES))],
                    ins=[cc_in[:].opt()], outs=[cc_out[:].opt()],
                )
                gath = smal.tile([128, 8, 2], f32, name=f"gath{idx}")
                src = cc_out[:].rearrange("(j p) v -> p j v", p=64)
                nc.sync.dma_start(gath[0:64], src)
                nc.sync.dma_start(gath[64:128], src)
                gs = smal.tile([128, 2], f32, name=f"gs{idx}")
                nc.vector.tensor_reduce(
                    gs[:], gath[:].rearrange("p j v -> p v j"),
                    axis=AX.X, op=ALU.add)

                # mean/var -> scale/shift (per partition, tiny ops)
                mv = smal.tile([128, 2], f32, name=f"mv{idx}")
                nc.vector.tensor_scalar_mul(mv[:], gs[:], 1.0 / NTOT)
                var = smal.tile([128, 1], f32, name=f"var{idx}")
                nc.vector.tensor_mul(var[:], mv[:, 0:1], mv[:, 0:1])
                nc.vector.tensor_sub(var[:], mv[:, 1:2], var[:])
                nc.vector.tensor_scalar_add(var[:], var[:], EPS)
                inv = smal.tile([128, 1], f32, name=f"inv{idx}")
                nc.vector.reciprocal(inv[:], var[:])
                istd = smal.tile([128, 1], f32, name=f"istd{idx}")
                nc.scalar.activation(istd[:], inv[:], AF.Sqrt)
                sc = smal.tile([128, 1], f32, name=f"sc{idx}")
                nc.vector.tensor_mul(sc[:], gb[:, 0:1], istd[:])
                sh = smal.tile([128, 1], f32, name=f"sh{idx}")
                nc.vector.tensor_mul(sh[:], mv[:, 0:1], sc[:])
                nc.vector.tensor_sub(sh[:], gb[:, 1:2], sh[:])
                return sc, sh

            # ---- conv1 -> BN1 stats sync -> relu(bn1) in place ----
            conv(x_sb, w1s, evict1)
            sc1, sh1 = bn_sync(sp1, gb1, 1)
            for k in range(2):
                zint = z_sb[:, :, 28 * k:28 * k + 28, 1:1 + W]
                nc.scalar.activation(zint, zint.bitcast(f32), AF.Relu,
                                     bias=sh1[:], scale=sc1[:])

            # ---- conv2 -> BN2 stats sync -> fused residual tail ----
            conv(z_sb, w2s, evict2)
            sc2, sh2 = bn_sync(sp2, gb2, 2)
            for j in range(2):
                for rb in range(0, H, 28):
                    y2g = y2[:, j, rb:rb + 28, :]
                    fing = fin[:, j, rb:rb + 28, :]
                    xg = x_sb[:, j, rb:rb + 28, 1:1 + W].bitcast(f32)
                    nc.vector.scalar_tensor_tensor(
                        fing, y2g, sc2[:], xg, op0=ALU.mult, op1=ALU.add)
                    nc.scalar.activation(y2g, fing, AF.Relu, bias=sh2[:])
                    for hh in range(2):
                        nc.sync.dma_start(
                            out_d[2 * hh + j, :, rb:rb + 28, :],
                            y2[64 * hh:64 * hh + 64, j, rb:rb + 28, :])

    nc.compile()
    return nc


_CACHE = {}


def _get_nc():
    if "nc" not in _CACHE:
        _CACHE["nc"] = build()
    return _CACHE["nc"]


def make_in_maps(x, w1, b1, g1, be1, w2, b2, g2, be2):
    """Shard + pre-pack host-side. Conv biases b1/b2 cancel exactly through
    the batch-norms (bn(x + c) == bn(x)), so they are dropped."""
    x = np.ascontiguousarray(np.asarray(x, np.float32))

    def packw(w):
        wt = np.ascontiguousarray(
            np.asarray(w, np.float32).transpose(2, 3, 1, 0).reshape(9, C, P))
        return np.ascontiguousarray(np.concatenate([wt, wt], axis=1))

    def packbn(g, be):
        g = np.asarray(g, np.float32)
        be = np.asarray(be, np.float32)
        return np.ascontiguousarray(
            np.stack([np.concatenate([g, g]), np.concatenate([be, be])], axis=1))

    w1p, w2p = packw(w1), packw(w2)
    bn1, bn2 = packbn(g1, be1), packbn(g2, be2)
    return [
        {"xs": np.ascontiguousarray(x[BL * r:BL * (r + 1)]),
         "w1p": w1p, "w2p": w2p, "bn1": bn1, "bn2": bn2}
        for r in range(N_CORES)
    ]


def kernel(x, w1, b1, g1, be1, w2, b2, g2, be2):
    nc = _get_nc()
    in_maps = make_in_maps(x, w1, b1, g1, be1, w2, b2, g2, be2)
    res = run_bass_kernel_spmd(nc, in_maps, core_ids=list(range(N_CORES)))
    return np.concatenate([res.results[r]["out"] for r in range(N_CORES)], axis=0)


if __name__ == "__main__":
    rng = np.random.default_rng(0)
    ins = {
        "x": rng.standard_normal((B, C, H, W)).astype(np.float32),
        "w1": rng.standard_normal((P, C, 3, 3)).astype(np.float32) * 0.04,
        "b1": rng.standard_normal((P,)).astype(np.float32) * 0.04,
        "g1": np.ones((P,), np.float32), "be1": np.zeros((P,), np.float32),
        "w2": rng.standard_normal((P, P, 3, 3)).astype(np.float32) * 0.04,
        "b2": rng.standard_normal((P,)).astype(np.float32) * 0.04,
        "g2": np.ones((P,), np.float32), "be2": np.zeros((P,), np.float32),
    }
    out = kernel(**ins)
    print("out", out.shape, out.dtype, float(np.abs(out).mean()))
